# revision 7
# baseline (speedup 1.0000x reference)
"""Multi-head linear attention on Trainium2 — 8-core SPMD, batch+head sharded.

Full-tensor contract: kernel(**inputs) takes the complete Q/K/V
[4, 4096, 1024] f32 arrays, internally shards them across 8 NeuronCores
(core c -> batch c//2, heads 8*(c%2) .. 8*(c%2)+8, i.e. a contiguous
512-column slice of the embedding dim), runs one Bass kernel per core,
and reassembles the full [4, 4096, 1024] f32 output.

Per-core math (H=8 local heads, D=64, L=4096):
    phi = sigmoid(0.6053*x - 4.102)
    kv_ext[h] = phi_K[h]^T @ [V[h] | 1]     # [64, 65], f32 PSUM accum
    numden[h] = phi_Q[h] @ kv_ext[h]        # [L, 65]
    out[h]    = numden[h][:, :64] / numden[h][:, 64:65]
Matmul inputs are bf16 (PSUM accumulation stays f32).
"""

import numpy as np

B = 4
L = 4096
E = 1024
NH = 8          # heads per core
D = 64
EC = NH * D     # 512 embedding columns per core
P = 128
NT = L // P     # 32 row tiles
TB = 4          # row tiles per DMA batch (1 MiB f32 loads)
NB = NT // TB
N_CORES = 8

_CACHE = {}


def _build_nc():
    from contextlib import ExitStack

    import concourse.bacc as bacc
    import concourse.mybir as mybir
    import concourse.tile as tile
    from concourse.masks import make_identity

    f32 = mybir.dt.float32
    bf16 = mybir.dt.bfloat16
    SIG = mybir.ActivationFunctionType.Sigmoid
    COPY = mybir.ActivationFunctionType.Copy

    nc = bacc.Bacc("TRN2", target_bir_lowering=False, debug=False)
    Q = nc.dram_tensor("Q", [L, EC], f32, kind="ExternalInput").ap()
    K = nc.dram_tensor("K", [L, EC], f32, kind="ExternalInput").ap()
    V = nc.dram_tensor("V", [L, EC], f32, kind="ExternalInput").ap()
    O = nc.dram_tensor("O", [L, EC], f32, kind="ExternalOutput").ap()

    with tile.TileContext(nc) as tc, ExitStack() as ctx:
        singles = ctx.enter_context(tc.tile_pool(name="singles", bufs=1))
        ld = ctx.enter_context(tc.tile_pool(name="ld", bufs=3))
        vb = ctx.enter_context(tc.tile_pool(name="vb", bufs=3))
        ph = ctx.enter_context(tc.tile_pool(name="ph", bufs=3))
        qt = ctx.enter_context(tc.tile_pool(name="qt", bufs=8))
        rcp = ctx.enter_context(tc.tile_pool(name="rcp", bufs=8))
        ob = ctx.enter_context(tc.tile_pool(name="ob", bufs=2))
        ident = singles.tile([P, P], bf16)
        make_identity(nc, ident)

        sig_bias = singles.tile([P, 1], f32)
        nc.vector.memset(sig_bias, -4.102)

        kv_sb = singles.tile([P, 4, D + 1], bf16)

        # ---- Phase K: kv_ext[h] = phi_K[h]^T @ [V[h] | 1] ----
        with tc.tile_pool(name="pk", bufs=1, space="PSUM") as pk:
            # 8 PSUM banks, one per head (one accumulation group per bank).
            # Head j lives on partitions (j%2)*64 .. +64 of its bank so the
            # phase-Q matmul finds it at the partition base it needs.
            kv_ps = [pk.tile([P, D + 1], f32, tag=f"kv{j}", name=f"kv{j}")
                     for j in range(NH)]
            for ib in range(NB):
                rows = slice(ib * TB * P, (ib + 1) * TB * P)
                k_raw = ld.tile([P, TB, EC], f32, tag="kraw", name="k_raw")
                nc.sync.dma_start(
                    out=k_raw,
                    in_=K[rows, :].rearrange("(t p) e -> p t e", p=P),
                )
                phiK = ph.tile([P, TB, EC], bf16, tag="phiK", name="phiK")
                nc.scalar.activation(
                    out=phiK, in_=k_raw, func=SIG, bias=sig_bias, scale=0.6053
                )
                v_bf = vb.tile([P, TB, NH, D + 1], bf16, name="v_bf")
                # SWDGE casts f32 -> bf16 in flight; column 64 of each head
                # block is set to 1.0 so one matmul also accumulates k_sum.
                for t in range(TB):
                    trows = slice((ib * TB + t) * P, (ib * TB + t + 1) * P)
                    nc.gpsimd.dma_start(
                        out=v_bf[:, t, :, 0:D],
                        in_=V[trows, :].rearrange("p (h d) -> p h d", h=NH),
                    )
                nc.vector.memset(v_bf[:, :, :, D:D + 1], 1.0)
                for t in range(TB):
                    for j in range(NH):
                        half = j % 2
                        nc.tensor.matmul(
                            out=kv_ps[j][half * D:(half + 1) * D, :],
                            lhsT=phiK[:, t, j * D:(j + 1) * D],
                            rhs=v_bf[:, t, j, :],
                            start=(ib == 0 and t == 0),
                            stop=(ib == NB - 1 and t == TB - 1),
                        )
            for j in range(NH):
                half = j % 2
                nc.vector.tensor_copy(
                    out=kv_sb[half * D:(half + 1) * D, j // 2, :],
                    in_=kv_ps[j][half * D:(half + 1) * D, :],
                )

        # PSUM pools for phase Q, opened after pk closes so banks are reused.
        pt = ctx.enter_context(tc.tile_pool(name="pt", bufs=2, space="PSUM"))
        pn = ctx.enter_context(tc.tile_pool(name="pn", bufs=2, space="PSUM"))

        # ---- Phase Q: numden = phi_Q @ kv_ext; out = num / den ----
        for ib in range(NB):
            rows = slice(ib * TB * P, (ib + 1) * TB * P)
            q_raw = ld.tile([P, TB, EC], f32, tag="qraw", name="q_raw")
            nc.sync.dma_start(
                out=q_raw,
                in_=Q[rows, :].rearrange("(t p) e -> p t e", p=P),
            )
            phiQ = ph.tile([P, TB, EC], bf16, tag="phiQ", name="phiQ")
            nc.scalar.activation(
                out=phiQ, in_=q_raw, func=SIG, bias=sig_bias, scale=0.6053
            )
            out_t = ob.tile([P, TB, EC], f32, name="out_t")
            for t in range(TB):
                for c in range(4):  # 128-col chunk == head pair (2c, 2c+1)
                    tp = pt.tile([P, P], bf16, tag="tp", name="tp")
                    nc.tensor.transpose(
                        out=tp, in_=phiQ[:, t, c * P:(c + 1) * P], identity=ident
                    )
                    qtT = qt.tile([P, P], bf16, tag="qtT", name="qtT")
                    nc.vector.tensor_copy(out=qtT, in_=tp)
                    # Two matmuls into one PSUM bank wedge the device on
                    # this stack, so each head gets its own bank.
                    nums = [pn.tile([P, D + 1], f32, tag=f"n{half}",
                                    name=f"n{half}") for half in range(2)]
                    for half in range(2):
                        nc.tensor.matmul(
                            out=nums[half],
                            lhsT=qtT[half * D:(half + 1) * D, :],
                            rhs=kv_sb[half * D:(half + 1) * D, c, :],
                        )
                    r = rcp.tile([P, 2], f32, tag="r", name="r")
                    for half in range(2):
                        nc.vector.reciprocal(
                            out=r[:, half:half + 1], in_=nums[half][:, D:D + 1])
                    for half in range(2):
                        j = 2 * c + half
                        nc.scalar.activation(
                            out=out_t[:, t, j * D:(j + 1) * D],
                            in_=nums[half][:, 0:D],
                            func=COPY,
                            scale=r[:, half:half + 1],
                        )
            nc.sync.dma_start(
                out=O[rows, :].rearrange("(t p) e -> p t e", p=P),
                in_=out_t,
            )

    nc.compile()
    return nc


def _get_nc():
    if "nc" not in _CACHE:
        _CACHE["nc"] = _build_nc()
    return _CACHE["nc"]


def _shard(arr):
    """Full [B, L, E] f32 -> list of 8 per-core [L, EC] slices."""
    out = []
    for c in range(N_CORES):
        b, g = divmod(c, 2)
        out.append(np.ascontiguousarray(arr[b, :, g * EC:(g + 1) * EC]))
    return out


def run_sharded(in_maps, trace=False, trace_cores=None):
    from concourse.bass_utils import run_bass_kernel_spmd

    nc = _get_nc()
    kwargs = {}
    if trace:
        kwargs = dict(trace=True, trace_cores=trace_cores or [0])
    return run_bass_kernel_spmd(nc, in_maps, core_ids=list(range(N_CORES)), **kwargs)


def kernel(**inputs):
    Q = np.ascontiguousarray(np.asarray(inputs["Q"], dtype=np.float32))
    K = np.ascontiguousarray(np.asarray(inputs["K"], dtype=np.float32))
    V = np.ascontiguousarray(np.asarray(inputs["V"], dtype=np.float32))
    qs, ks, vs = _shard(Q), _shard(K), _shard(V)
    in_maps = [{"Q": qs[c], "K": ks[c], "V": vs[c]} for c in range(N_CORES)]
    res = run_sharded(in_maps)
    out = np.empty((B, L, E), dtype=np.float32)
    for c in range(N_CORES):
        b, g = divmod(c, 2)
        out[b, :, g * EC:(g + 1) * EC] = res.results[c]["O"]
    return out


# revision 8
# speedup vs baseline: 1.3385x; 1.3385x over previous
"""Multi-head linear attention on Trainium2 — 8-core SPMD, batch+head sharded.

Full-tensor contract: kernel(**inputs) takes the complete Q/K/V
[4, 4096, 1024] f32 arrays, internally shards them across 8 NeuronCores
(core c -> batch c//2, heads 8*(c%2) .. 8*(c%2)+8, i.e. a contiguous
512-column slice of the embedding dim), runs one Bass kernel per core,
and reassembles the full [4, 4096, 1024] f32 output.

Per-core math (H=8 local heads, D=64, L=4096):
    phi = sigmoid(0.6053*x - 4.102)
    kv_ext[h] = phi_K[h]^T @ [V[h] | 1]     # [64, 65], f32 PSUM accum
    numden[h] = phi_Q[h] @ kv_ext[h]        # [L, 65]
    out[h]    = numden[h][:, :64] / numden[h][:, 64:65]

Heads are processed in pairs: one K=128 matmul per head pair computes
both heads' kv_ext blocks (phi_K pair chunk as stationary, [V|1] pair
as moving; off-diagonal blocks are discarded), and one K=128 matmul per
pair computes both numden blocks against a block-diagonal kv operand.
Matmul inputs are bf16 (PSUM accumulation stays f32).
"""

import numpy as np

B = 4
L = 4096
E = 1024
NH = 8          # heads per core
D = 64
W = D + 1       # head block width incl. ones/den column
EC = NH * D     # 512 embedding columns per core
P = 128
NT = L // P     # 32 row tiles
TB = 4          # row tiles per DMA batch (1 MiB f32 loads)
NB = NT // TB
N_CORES = 8

_CACHE = {}


def _build_nc():
    from contextlib import ExitStack

    import concourse.bacc as bacc
    import concourse.bass as bass
    import concourse.mybir as mybir
    import concourse.tile as tile
    from concourse.masks import make_identity

    f32 = mybir.dt.float32
    bf16 = mybir.dt.bfloat16
    SIG = mybir.ActivationFunctionType.Sigmoid

    nc = bacc.Bacc("TRN2", target_bir_lowering=False, debug=False)
    Q = nc.dram_tensor("Q", [L, EC], f32, kind="ExternalInput").ap()
    K = nc.dram_tensor("K", [L, EC], f32, kind="ExternalInput").ap()
    V = nc.dram_tensor("V", [L, EC], f32, kind="ExternalInput").ap()
    O = nc.dram_tensor("O", [L, EC], f32, kind="ExternalOutput").ap()

    with tile.TileContext(nc) as tc, ExitStack() as ctx:
        singles = ctx.enter_context(tc.tile_pool(name="singles", bufs=1))
        ld = ctx.enter_context(tc.tile_pool(name="ld", bufs=3))
        vb = ctx.enter_context(tc.tile_pool(name="vb", bufs=3))
        ph = ctx.enter_context(tc.tile_pool(name="ph", bufs=3))
        qt = ctx.enter_context(tc.tile_pool(name="qt", bufs=8))
        rcp = ctx.enter_context(tc.tile_pool(name="rcp", bufs=8))
        ob = ctx.enter_context(tc.tile_pool(name="ob", bufs=2))

        ident = singles.tile([P, P], f32)
        make_identity(nc, ident)

        sig_bias = singles.tile([P, 1], f32)
        nc.vector.memset(sig_bias, -4.102)

        # Block-diagonal kv operand per head pair: rows 0:64 cols 0:65 hold
        # kv_ext of the even head, rows 64:128 cols 65:130 the odd head.
        kv_bd = singles.tile([P, 4, 2 * W], bf16)
        nc.vector.memset(kv_bd, 0.0)

        # ---- Phase K: per pair, kv_pair = phiK_pair^T @ [V|1]_pair ----
        with tc.tile_pool(name="pk", bufs=1, space="PSUM") as pk:
            kv_ps = [pk.tile([P, 2 * W], f32, tag=f"kv{c}", name=f"kv{c}")
                     for c in range(4)]
            for ib in range(NB):
                rows = slice(ib * TB * P, (ib + 1) * TB * P)
                k_raw = ld.tile([P, TB, EC], f32, tag="kraw", name="k_raw")
                nc.sync.dma_start(
                    out=k_raw,
                    in_=K[rows, :].rearrange("(t p) e -> p t e", p=P),
                )
                phiK = ph.tile([P, TB, EC], bf16, tag="phiK", name="phiK")
                nc.scalar.activation(
                    out=phiK, in_=k_raw, func=SIG, bias=sig_bias, scale=0.6053
                )
                v_bf = vb.tile([P, TB, NH, W], bf16, name="v_bf")
                # SWDGE casts f32 -> bf16 in flight; column 64 of each head
                # block is set to 1.0 so the matmul also accumulates k_sum.
                for t in range(TB):
                    trows = slice((ib * TB + t) * P, (ib * TB + t + 1) * P)
                    nc.gpsimd.dma_start(
                        out=v_bf[:, t, :, 0:D],
                        in_=V[trows, :].rearrange("p (h d) -> p h d", h=NH),
                    )
                nc.vector.memset(v_bf[:, :, :, D:W], 1.0)
                for t in range(TB):
                    for c in range(4):
                        nc.tensor.matmul(
                            out=kv_ps[c],
                            lhsT=phiK[:, t, c * P:(c + 1) * P],
                            rhs=v_bf[:, t, 2 * c:2 * c + 2, :],
                            start=(ib == 0 and t == 0),
                            stop=(ib == NB - 1 and t == TB - 1),
                        )
            for c in range(4):
                nc.vector.tensor_copy(
                    out=kv_bd[0:D, c, 0:W], in_=kv_ps[c][0:D, 0:W])
                nc.vector.tensor_copy(
                    out=kv_bd[D:P, c, W:2 * W], in_=kv_ps[c][D:P, W:2 * W])

        # PSUM pools for phase Q, opened after pk closes.
        pt = ctx.enter_context(tc.tile_pool(name="pt", bufs=4, space="PSUM"))
        pn = ctx.enter_context(tc.tile_pool(name="pn", bufs=4, space="PSUM"))

        # ---- Phase Q: transpose raw Q on PE, sigmoid PSUM->SBUF on ACT,
        # one matmul per pair against block-diagonal kv, divide on DVE ----
        for ib in range(NB):
            rows = slice(ib * TB * P, (ib + 1) * TB * P)
            q_raw = ld.tile([P, TB, EC], f32, tag="qraw", name="q_raw")
            nc.sync.dma_start(
                out=q_raw,
                in_=Q[rows, :].rearrange("(t p) e -> p t e", p=P),
            )
            out_t = ob.tile([P, TB, EC], f32, name="out_t")
            for t in range(TB):
                for c in range(4):  # 128-col chunk == head pair (2c, 2c+1)
                    tp = pt.tile([P, P], f32, tag="tp", name="tp")
                    nc.tensor.transpose(
                        out=tp, in_=q_raw[:, t, c * P:(c + 1) * P], identity=ident
                    )
                    qtT = qt.tile([P, P], bf16, tag="qtT", name="qtT")
                    nc.scalar.activation(
                        out=qtT, in_=tp, func=SIG, bias=sig_bias, scale=0.6053
                    )
                    num = pn.tile([P, 2, W], f32, tag="num", name="num")
                    nc.tensor.matmul(
                        out=num.rearrange("p a b -> p (a b)"),
                        lhsT=qtT,
                        rhs=kv_bd[:, c, :],
                    )
                    r = rcp.tile([P, 2], f32, tag="r", name="r")
                    nc.vector.reciprocal(out=r, in_=num[:, :, D])
                    r_bc = bass.AP(
                        tensor=r.tensor, offset=r.offset,
                        ap=[r.ap[0], r.ap[1], [0, D]],
                    )
                    nc.vector.tensor_tensor(
                        out=out_t[:, t, c * P:(c + 1) * P].rearrange(
                            "p (a d) -> p a d", a=2),
                        in0=num[:, :, 0:D],
                        in1=r_bc,
                        op=mybir.AluOpType.mult,
                    )
            nc.sync.dma_start(
                out=O[rows, :].rearrange("(t p) e -> p t e", p=P),
                in_=out_t,
            )

    nc.compile()
    return nc


def _get_nc():
    if "nc" not in _CACHE:
        _CACHE["nc"] = _build_nc()
    return _CACHE["nc"]


def _shard(arr):
    """Full [B, L, E] f32 -> list of 8 per-core [L, EC] slices."""
    out = []
    for c in range(N_CORES):
        b, g = divmod(c, 2)
        out.append(np.ascontiguousarray(arr[b, :, g * EC:(g + 1) * EC]))
    return out


def run_sharded(in_maps, trace=False, trace_cores=None):
    from concourse.bass_utils import run_bass_kernel_spmd

    nc = _get_nc()
    kwargs = {}
    if trace:
        kwargs = dict(trace=True, trace_cores=trace_cores or [0])
    return run_bass_kernel_spmd(nc, in_maps, core_ids=list(range(N_CORES)), **kwargs)


def kernel(**inputs):
    Q = np.ascontiguousarray(np.asarray(inputs["Q"], dtype=np.float32))
    K = np.ascontiguousarray(np.asarray(inputs["K"], dtype=np.float32))
    V = np.ascontiguousarray(np.asarray(inputs["V"], dtype=np.float32))
    qs, ks, vs = _shard(Q), _shard(K), _shard(V)
    in_maps = [{"Q": qs[c], "K": ks[c], "V": vs[c]} for c in range(N_CORES)]
    res = run_sharded(in_maps)
    out = np.empty((B, L, E), dtype=np.float32)
    for c in range(N_CORES):
        b, g = divmod(c, 2)
        out[b, :, g * EC:(g + 1) * EC] = res.results[c]["O"]
    return out
